# revision 1
# baseline (speedup 1.0000x reference)
"""Local (Gaussian-banded) attention kernel for Trainium2, 8 NeuronCores.

Math: out = rownorm(gauss_band(sigma)) @ (x @ Wg) @ Wout
The Gaussian positional mask with sigma in [0.5, 2.5] decays below fp32
resolution past |i-j| > 32, so attention is a 65-tap banded matmul.

Sharding: core c = (batch b = c//2, seq-half s = c%2). Each core computes
out rows [s*1024, (s+1)*1024) of its batch. s=1 halves are row-reversed on
host so the sequence edge is always at local row 0 -> all 8 cores run the
same program with the same band constants (pure SPMD).

Per-core device pipeline (fp32 data, float32r matmuls = full-rate PE):
  stage 1: v[1088,1024] = xpad @ Wg          (lhsT = host-transposed xT)
  stage 2: banded attention in 256-row chunks; for chunk i the 320-row
           source window spans v tiles 2i (A), 2i+1 (B), 2i+2[:64] (C);
           band matrices (moving operand, N=256) carry interior row-norm;
           the 32 sequence-edge rows are fixed by a tiny column rescale.
  stage 3: out[1024,512] = attn @ Wout       (lhsT = attnT, accumulate heads)
"""

import sys

for _p in ("/opt/trn_rl_repo", "/root/.axon_site/_ro/trn_rl_repo"):
    if _p not in sys.path:
        sys.path.append(_p)

import numpy as np

B, N, D = 4, 2048, 512
H, DH = 8, 128
INNER = H * DH
W = 32                      # band half-width
VROWS = 1088                # 32 zero pad + 1024 own + 32 halo
NT = 9                      # v tiles: 8 x 128 + 1 x 64
USE_F32R = True             # float32r matmuls (4x faster, ~2e-4 rel err)

_CACHE = {}


def _build_nc(debug_outputs=False):
    import concourse.mybir as mybir
    from concourse import bacc
    from concourse.tile import TileContext

    f32 = mybir.dt.float32
    mdt = mybir.dt.float32r if USE_F32R else f32

    def mm(out, lhsT, rhs, **kw):
        nc.tensor.matmul(out, lhsT, rhs, **kw)

    nc = bacc.Bacc(None, target_bir_lowering=False)

    xT = nc.dram_tensor("xT", [D, VROWS], mdt, kind="ExternalInput")
    wg = nc.dram_tensor("Wg", [D, INNER], mdt, kind="ExternalInput")
    wout = nc.dram_tensor("Wout", [INNER, D], mdt, kind="ExternalInput")
    # band pieces: [m, (h, r)] with r = 256 chunk cols; C only has 64 rows
    bandA = nc.dram_tensor("bandA", [128, H * 256], mdt, kind="ExternalInput")
    bandB = nc.dram_tensor("bandB", [128, H * 256], mdt, kind="ExternalInput")
    bandC = nc.dram_tensor("bandC", [64, H * 256], mdt, kind="ExternalInput")
    # edge rescale for out rows 0..31: [128(bcast), (h, 32)]
    ec = nc.dram_tensor("ec", [128, H * 32], mdt, kind="ExternalInput")
    out = nc.dram_tensor("out", [1024, D], f32, kind="ExternalOutput")
    if debug_outputs:
        v_dbg = nc.dram_tensor("v_dbg", [NT * 128, INNER], mdt, kind="ExternalOutput")
        attnT_dbg = nc.dram_tensor("attnT_dbg", [128, 8 * H * 128], mdt,
                                   kind="ExternalOutput")

    with TileContext(nc) as tc:
        with (
            tc.tile_pool(name="persist", bufs=1) as pp,
            tc.tile_pool(name="stage", bufs=2) as sp,
            tc.tile_pool(name="ps1", bufs=3, space="PSUM") as ps1,
            tc.tile_pool(name="ps2", bufs=3, space="PSUM") as ps2,
            tc.tile_pool(name="ps3", bufs=2, space="PSUM") as ps3,
        ):
            # ---- input DMAs, chunked + ordered so stage-1 starts early:
            # xT cols 0:256 -> wg half0 -> wg half1 -> rest of xT -> bands -> Wout
            xT_sb = pp.tile([128, 4 * VROWS], mdt, tag="xT", name="xT_sb")
            wg_sb = pp.tile([128, 4 * INNER], mdt, tag="wg", name="wg_sb")
            xT_d = xT.rearrange("(k p) c -> p k c", p=128)
            xT_s = xT_sb.rearrange("p (k c) -> p k c", c=VROWS)
            wg_d = wg.rearrange("(k p) c -> p k c", p=128)
            wg_s = wg_sb.rearrange("p (k c) -> p k c", c=INNER)
            nc.sync.dma_start(out=xT_s[:, :, 0:256], in_=xT_d[:, :, 0:256])
            nc.sync.dma_start(out=wg_s[:, :, 0:512], in_=wg_d[:, :, 0:512])
            nc.sync.dma_start(out=wg_s[:, :, 512:1024], in_=wg_d[:, :, 512:1024])
            nc.sync.dma_start(out=xT_s[:, :, 256:640], in_=xT_d[:, :, 256:640])
            nc.sync.dma_start(out=xT_s[:, :, 640:VROWS], in_=xT_d[:, :, 640:VROWS])
            bandA_sb = pp.tile([128, H * 256], mdt, tag="bandA", name="bandA_sb")
            nc.sync.dma_start(out=bandA_sb, in_=bandA[:, :])
            bandB_sb = pp.tile([128, H * 256], mdt, tag="bandB", name="bandB_sb")
            nc.sync.dma_start(out=bandB_sb, in_=bandB[:, :])
            bandC_sb = pp.tile([64, H * 256], mdt, tag="bandC", name="bandC_sb")
            nc.sync.dma_start(out=bandC_sb, in_=bandC[:, :])
            ec_sb = pp.tile([128, H * 32], mdt, tag="ec", name="ec_sb")
            nc.sync.dma_start(out=ec_sb, in_=ec[:, :])
            wout_sb = pp.tile([128, H * D], mdt, tag="wout", name="wout_sb")
            nc.sync.dma_start(
                out=wout_sb.rearrange("p (h c) -> p h c", c=D),
                in_=wout.rearrange("(h p) c -> p h c", p=128))

            v_sb = [pp.tile([128, INNER], mdt, tag=f"v{t}", name=f"v{t}")
                    for t in range(NT)]
            # attnT layout: [dh(128), (i(4), h(8), r(256))]
            attnT = pp.tile([128, 4 * H * 256], mdt, tag="attnT", name="attnT")

            # ---- stage 1: v tiles = xpad @ Wg
            for t in range(NT):
                rows = 128 if t < 8 else 64
                psA = ps1.tile([128, 512], f32, tag="s1", name=f"psA{t}")
                psB = ps1.tile([128, 512], f32, tag="s1", name=f"psB{t}")
                for k in range(4):
                    lh = xT_sb[:, k * VROWS + t * 128: k * VROWS + t * 128 + rows]
                    mm(psA[:rows, :], lh, wg_sb[:, k * INNER:k * INNER + 512],
                       start=(k == 0), stop=(k == 3))
                    mm(psB[:rows, :], lh, wg_sb[:, k * INNER + 512:(k + 1) * INNER],
                       start=(k == 0), stop=(k == 3))
                nc.vector.tensor_copy(v_sb[t][:rows, 0:512], psA[:rows, :])
                nc.vector.tensor_copy(v_sb[t][:rows, 512:1024], psB[:rows, :])
                if debug_outputs:
                    nc.sync.dma_start(out=v_dbg[t * 128:t * 128 + rows, :],
                                      in_=v_sb[t][:rows, :])

            # ---- stage 2: banded attention -> attnT (4 sweeps of 2 heads)
            # chunk i (256 out rows): A = v[2i] (K=128), B = v[2i+1] (K=128),
            # C = v[2i+2][:64] (K=64). One psum tile [128,512] = 2 heads.
            # PSUM has_written clear is whole-bank on start=True: only the
            # FIRST matmul into each psum tile carries start=True.
            for g in range(4):
                hs = (2 * g, 2 * g + 1)
                ps_chunk = {}
                for t in range(NT):
                    for hh, h in enumerate(hs):
                        hsl = slice(h * 256, (h + 1) * 256)
                        csl = slice(hh * 256, (hh + 1) * 256)
                        if t % 2 == 0:
                            i_new, i_fin = t // 2, t // 2 - 1
                            if i_new <= 3:
                                if hh == 0:
                                    ps_chunk[i_new] = ps2.tile(
                                        [128, 512], f32, tag="s2",
                                        name=f"ps2_{g}_{i_new}")
                                mm(ps_chunk[i_new][:, csl],
                                   v_sb[t][:, h * 128:(h + 1) * 128],
                                   bandA_sb[:, hsl], start=(hh == 0), stop=False)
                            if i_fin >= 0:
                                mm(ps_chunk[i_fin][:, csl],
                                   v_sb[t][:64, h * 128:(h + 1) * 128],
                                   bandC_sb[:, hsl], start=False, stop=(hh == 1))
                        else:
                            i_mid = (t - 1) // 2
                            mm(ps_chunk[i_mid][:, csl],
                               v_sb[t][:, h * 128:(h + 1) * 128],
                               bandB_sb[:, hsl], start=False, stop=False)
                    if t % 2 == 0 and t // 2 - 1 >= 0:
                        i_fin = t // 2 - 1
                        nc.vector.tensor_copy(
                            attnT[:, (i_fin * 8 + 2 * g) * 256:
                                  (i_fin * 8 + 2 * g + 2) * 256],
                            ps_chunk.pop(i_fin))
                        if i_fin == 0:
                            # edge rescale: first 32 out rows of the sequence
                            blk = attnT[:, 2 * g * 256:(2 * g + 2) * 256]
                            v3 = blk.rearrange("p (h r) -> p h r", r=256)[:, :, 0:32]
                            nc.vector.tensor_mul(
                                v3, v3,
                                ec_sb[:, 2 * g * 32:(2 * g + 2) * 32].rearrange(
                                    "p (h r) -> p h r", r=32))

            if debug_outputs:
                at = attnT.rearrange("p (i h r) -> p i h r", i=4, h=H)
                ad = attnT_dbg.rearrange("p (i h r) -> p i h r", i=8, h=H)
                for i in range(4):
                    for half in range(2):
                        nc.sync.dma_start(
                            out=ad[:, 2 * i + half, :, :],
                            in_=at[:, i, :, half * 128:half * 128 + 128])

            # ---- stage 3: out chunks = attn @ Wout (accumulate over heads)
            ot = {}
            for j in range(8):
                i, half = j // 2, j % 2
                ps = ps3.tile([128, 512], f32, tag="s3", name=f"ps3_{j}")
                for h in range(H):
                    off = (i * 8 + h) * 256 + half * 128
                    mm(ps, attnT[:, off:off + 128],
                       wout_sb[:, h * D:(h + 1) * D],
                       start=(h == 0), stop=(h == 7))
                if j % 4 == 0:
                    ot[j // 4] = sp.tile([128, 4 * D], f32, tag="outt",
                                         name=f"ot{j // 4}")
                nc.vector.tensor_copy(ot[j // 4][:, (j % 4) * D:(j % 4 + 1) * D], ps)
                if j % 4 == 3:
                    nc.sync.dma_start(
                        out=out[(j - 3) * 128:(j + 1) * 128, :].rearrange(
                            "(c p) d -> p c d", p=128),
                        in_=ot[j // 4].rearrange("p (c d) -> p c d", d=D))

    nc.compile()
    return nc


def _band_constants(sigma: np.ndarray):
    """Band pieces (interior row-norm baked in) + edge rescale, float64 host."""
    sig = np.asarray(sigma, np.float64).reshape(H)
    d = np.arange(W + 1, dtype=np.float64)
    wts = np.exp(-(d[None, :] ** 2) / (2.0 * sig[:, None] ** 2))  # [H, 33]
    tail = wts[:, 1:].sum(1)
    s_int = wts[:, 0] + 2.0 * tail
    # edge rowsum for out rows r=0..31 (left-truncated gaussian)
    re = np.arange(32)
    cum = np.concatenate([np.zeros((H, 1)), np.cumsum(wts[:, 1:], 1)], 1)
    s_edge = wts[:, [0]] + cum[:, np.minimum(re, W)] + tail[:, None]  # [H, 32]

    r = np.arange(256)

    def piece(m_count, m_off):
        mloc = np.arange(m_count)
        dist = np.abs(r[None, :] + 32 - (m_off + mloc[:, None]))
        msk = dist <= W
        wp = np.where(msk[None], wts[:, np.minimum(dist, W).astype(int)], 0.0)
        wp = wp / s_int[:, None, None]                    # [H, m, 256]
        return np.ascontiguousarray(
            wp.transpose(1, 0, 2).reshape(m_count, H * 256)).astype(np.float32)

    bandA = piece(128, 0)
    bandB = piece(128, 128)
    bandC = piece(64, 256)
    ecv = (s_int[:, None] / s_edge).astype(np.float32)    # [H, 32]
    ec = np.ascontiguousarray(
        np.broadcast_to(ecv.reshape(1, H * 32), (128, H * 32))).astype(np.float32)
    return bandA, bandB, bandC, ec


def _in_maps(x, Wg, Wout, sigma):
    bandA, bandB, bandC, ec = _band_constants(sigma)
    wg = np.ascontiguousarray(np.asarray(Wg, np.float32))
    wo = np.ascontiguousarray(np.asarray(Wout, np.float32))
    x = np.asarray(x, np.float32)
    maps = []
    for c in range(8):
        b, s = divmod(c, 2)
        z = x[b] if s == 0 else x[b, ::-1]
        xbuf = np.zeros((VROWS, D), np.float32)
        xbuf[32:] = z[:1056]
        maps.append({
            "xT": np.ascontiguousarray(xbuf.T),
            "Wg": wg, "Wout": wo,
            "bandA": bandA, "bandB": bandB, "bandC": bandC, "ec": ec,
        })
    return maps


def _get_nc():
    if "nc" not in _CACHE:
        _CACHE["nc"] = _build_nc()
    return _CACHE["nc"]


def run_spmd(in_maps, **kw):
    from concourse.bass_utils import run_bass_kernel_spmd
    return run_bass_kernel_spmd(_get_nc(), in_maps, core_ids=list(range(8)), **kw)


def _assemble(results):
    full = np.empty((B, N, D), np.float32)
    for c in range(8):
        b, s = divmod(c, 2)
        r = results[c]["out"]
        if s == 0:
            full[b, :1024] = r
        else:
            full[b, 1024:] = r[::-1]
    return full


def kernel(x, Wg, Wout, sigma):
    res = run_spmd(_in_maps(x, Wg, Wout, sigma))
    return _assemble(res.results)



# revision 2
# speedup vs baseline: 1.0857x; 1.0857x over previous
"""Local (Gaussian-banded) attention kernel for Trainium2, 8 NeuronCores.

Math: out = rownorm(gauss_band(sigma)) @ (x @ Wg) @ Wout
The Gaussian positional mask with sigma in [0.5, 2.5] decays below fp32
resolution past |i-j| > 32, so attention is a 65-tap banded matmul.

Sharding: core c = (batch b = c//2, seq-half s = c%2). Each core computes
out rows [s*1024, (s+1)*1024) of its batch. s=1 halves are row-reversed on
host so the sequence edge is always at local row 0 -> all 8 cores run the
same program with the same band constants (pure SPMD).

v2: all-bf16 datapath (fp32 PSUM accumulate). bf16 enables fast weight
load (FWL) so LDWEIGHTS no longer paces the band matmuls, halves DMA
bytes and PSUM->SBUF cast time. Host prepacks every tensor into its
SBUF layout so each DMA is contiguous >=2KB per partition. Stages are
interleaved chunk-major (256 out rows per chunk): stage-2 band matmuls
and stage-3 output projection for chunk i run while stage 1 still
produces v tiles for later chunks, and each chunk's output DMA streams
out under compute. Measured rel err vs fp32 reference ~4.5e-3.

Per-core device pipeline:
  stage 1: v[1088,1024] = xpad @ Wg          (lhsT = host-transposed xT)
  stage 2: banded attention chunk i: psum[dh=128, (h,256)] accumulates
           bandA (v tile 2i), bandB (2i+1), bandC (2i+2 rows :64);
           interior row-norm baked into band matrices; 32 sequence-edge
           rows fixed by a column rescale after the PSUM->SBUF cast.
  stage 3: out[256,512] = attn chunk @ Wout  (accumulate over heads)
"""

import sys

for _p in ("/opt/trn_rl_repo", "/root/.axon_site/_ro/trn_rl_repo"):
    if _p not in sys.path:
        sys.path.append(_p)

import numpy as np
import ml_dtypes

BF16 = ml_dtypes.bfloat16

B, N, D = 4, 2048, 512
H, DH = 8, 128
INNER = H * DH
W = 32                      # band half-width
VROWS = 1088                # 32 zero pad + 1024 own + 32 halo
NT = 9                      # v tiles: 8 x 128 + 1 x 64

_CACHE = {}


def _build_nc(debug_outputs=False):
    import concourse.mybir as mybir
    from concourse import bacc
    from concourse.tile import TileContext

    f32 = mybir.dt.float32
    bf = mybir.dt.bfloat16

    nc = bacc.Bacc(None, target_bir_lowering=False)

    def mm(out, lhsT, rhs, **kw):
        nc.tensor.matmul(out, lhsT, rhs, **kw)

    # all inputs prepacked on host into SBUF layout, bf16
    xT = nc.dram_tensor("xT", [128, 4 * VROWS], bf, kind="ExternalInput")
    wg = nc.dram_tensor("Wg", [128, 4 * INNER], bf, kind="ExternalInput")
    wout = nc.dram_tensor("Wout", [128, H * D], bf, kind="ExternalInput")
    bandA = nc.dram_tensor("bandA", [128, H * 256], bf, kind="ExternalInput")
    bandB = nc.dram_tensor("bandB", [128, H * 256], bf, kind="ExternalInput")
    bandC = nc.dram_tensor("bandC", [64, H * 256], bf, kind="ExternalInput")
    ec = nc.dram_tensor("ec", [128, H * 32], bf, kind="ExternalInput")
    # out rows r = i*256 + half*128 + p  ->  cols i*1024 + half*512 + d
    out = nc.dram_tensor("out", [128, 4 * 1024], bf, kind="ExternalOutput")

    with TileContext(nc) as tc:
        with (
            tc.tile_pool(name="persist", bufs=1) as pp,
            tc.tile_pool(name="attn", bufs=2) as ap,
            tc.tile_pool(name="outs", bufs=2) as osp,
            tc.tile_pool(name="ps1", bufs=2, space="PSUM") as ps1,
            tc.tile_pool(name="ps2", bufs=4, space="PSUM") as ps2,
            tc.tile_pool(name="ps3", bufs=2, space="PSUM") as ps3,
        ):
            xT_sb = pp.tile([128, 4 * VROWS], bf, tag="xT", name="xT_sb")
            wg_sb = pp.tile([128, 4 * INNER], bf, tag="wg", name="wg_sb")
            xT_s = xT_sb.rearrange("p (k c) -> p k c", c=VROWS)
            xT_d = xT.rearrange("p (k c) -> p k c", c=VROWS)
            wg_s = wg_sb.rearrange("p (k c) -> p k c", c=INNER)
            wg_d = wg.rearrange("p (k c) -> p k c", c=INNER)
            # interleave per-k so stage-1 k-chunk MMs start as data lands;
            # xT on sync queue, everything else on scalar queue (parallel issue)
            for k in range(4):
                nc.sync.dma_start(out=xT_s[:, k, :], in_=xT_d[:, k, :])
                nc.scalar.dma_start(out=wg_s[:, k, :], in_=wg_d[:, k, :])
            bandA_sb = pp.tile([128, H * 256], bf, tag="bandA", name="bandA_sb")
            bandB_sb = pp.tile([128, H * 256], bf, tag="bandB", name="bandB_sb")
            bandC_sb = pp.tile([64, H * 256], bf, tag="bandC", name="bandC_sb")
            ec_sb = pp.tile([128, H * 32], bf, tag="ec", name="ec_sb")
            wout_sb = pp.tile([128, H * D], bf, tag="wout", name="wout_sb")
            nc.scalar.dma_start(out=bandA_sb, in_=bandA[:, :])
            nc.scalar.dma_start(out=bandB_sb, in_=bandB[:, :])
            nc.scalar.dma_start(out=bandC_sb, in_=bandC[:, :])
            nc.scalar.dma_start(out=ec_sb, in_=ec[:, :])
            nc.scalar.dma_start(out=wout_sb, in_=wout[:, :])

            v_sb = [pp.tile([128, INNER], bf, tag=f"v{t}", name=f"v{t}")
                    for t in range(NT)]

            def s1(t):
                rows = 128 if t < 8 else 64
                psA = ps1.tile([128, 512], f32, tag="s1", name=f"psA{t}")
                psB = ps1.tile([128, 512], f32, tag="s1", name=f"psB{t}")
                for k in range(4):
                    lh = xT_sb[:, k * VROWS + t * 128: k * VROWS + t * 128 + rows]
                    mm(psA[:rows, :], lh, wg_sb[:, k * INNER:k * INNER + 512],
                       start=(k == 0), stop=(k == 3))
                for k in range(4):
                    lh = xT_sb[:, k * VROWS + t * 128: k * VROWS + t * 128 + rows]
                    mm(psB[:rows, :], lh, wg_sb[:, k * INNER + 512:(k + 1) * INNER],
                       start=(k == 0), stop=(k == 3))
                nc.vector.tensor_copy(v_sb[t][:rows, 0:512], psA[:rows, :])
                nc.vector.tensor_copy(v_sb[t][:rows, 512:1024], psB[:rows, :])

            attn_c = {}

            def s2(i):
                # attnT chunk i: [dh=128, (h, r=256)] bf16
                at = ap.tile([128, H * 256], bf, tag="attnT", name=f"attnT{i}")
                attn_c[i] = at
                vA, vB, vC = v_sb[2 * i], v_sb[2 * i + 1], v_sb[2 * i + 2]
                for p4 in range(4):
                    ps = ps2.tile([128, 512], f32, tag="s2", name=f"ps2_{i}_{p4}")
                    for hh in range(2):
                        h = 2 * p4 + hh
                        hsl = slice(h * 256, (h + 1) * 256)
                        csl = slice(hh * 256, (hh + 1) * 256)
                        mm(ps[:, csl], vA[:, h * 128:(h + 1) * 128],
                           bandA_sb[:, hsl], start=(hh == 0), stop=False)
                        mm(ps[:, csl], vB[:, h * 128:(h + 1) * 128],
                           bandB_sb[:, hsl], start=False, stop=False)
                        mm(ps[:, csl], vC[:64, h * 128:(h + 1) * 128],
                           bandC_sb[:, hsl], start=False, stop=(hh == 1))
                    nc.vector.tensor_copy(at[:, p4 * 512:(p4 + 1) * 512], ps)
                if i == 0:
                    # edge rescale: first 32 sequence rows, all heads at once
                    v3 = at.rearrange("p (h r) -> p h r", r=256)[:, :, 0:32]
                    nc.vector.tensor_mul(
                        v3, v3, ec_sb.rearrange("p (h r) -> p h r", r=32))

            def s3(i):
                at = attn_c.pop(i)
                ot = osp.tile([128, 1024], bf, tag="outt", name=f"ot{i}")
                for half in range(2):
                    ps = ps3.tile([128, 512], f32, tag="s3", name=f"ps3_{i}_{half}")
                    for h in range(H):
                        off = h * 256 + half * 128
                        mm(ps, at[:, off:off + 128],
                           wout_sb[:, h * D:(h + 1) * D],
                           start=(h == 0), stop=(h == 7))
                    nc.vector.tensor_copy(ot[:, half * 512:(half + 1) * 512], ps)
                nc.scalar.dma_start(out=out[:, i * 1024:(i + 1) * 1024], in_=ot)

            # interleaved schedule: keep PE dense, stream outputs early
            s1(0); s1(1); s1(2); s1(3)
            s2(0)
            s1(4)
            s3(0)
            s2(1)
            s1(5); s1(6)
            s3(1)
            s2(2)
            s1(7); s1(8)
            s3(2)
            s2(3)
            s3(3)

    nc.compile()
    return nc


def _band_constants(sigma: np.ndarray):
    """Band pieces (interior row-norm baked in) + edge rescale, float64 host."""
    sig = np.asarray(sigma, np.float64).reshape(H)
    d = np.arange(W + 1, dtype=np.float64)
    wts = np.exp(-(d[None, :] ** 2) / (2.0 * sig[:, None] ** 2))  # [H, 33]
    tail = wts[:, 1:].sum(1)
    s_int = wts[:, 0] + 2.0 * tail
    # edge rowsum for out rows r=0..31 (left-truncated gaussian)
    re = np.arange(32)
    cum = np.concatenate([np.zeros((H, 1)), np.cumsum(wts[:, 1:], 1)], 1)
    s_edge = wts[:, [0]] + cum[:, np.minimum(re, W)] + tail[:, None]  # [H, 32]

    r = np.arange(256)

    def piece(m_count, m_off):
        mloc = np.arange(m_count)
        dist = np.abs(r[None, :] + 32 - (m_off + mloc[:, None]))
        msk = dist <= W
        wp = np.where(msk[None], wts[:, np.minimum(dist, W).astype(int)], 0.0)
        wp = wp / s_int[:, None, None]                    # [H, m, 256]
        return np.ascontiguousarray(
            wp.transpose(1, 0, 2).reshape(m_count, H * 256)).astype(BF16)

    bandA = piece(128, 0)
    bandB = piece(128, 128)
    bandC = piece(64, 256)
    ecv = (s_int[:, None] / s_edge).astype(np.float32)    # [H, 32]
    ecb = np.ascontiguousarray(
        np.broadcast_to(ecv.reshape(1, H * 32), (128, H * 32))).astype(BF16)
    return bandA, bandB, bandC, ecb


def _pack_k(a, cols):
    # [512, cols] -> [128, 4*cols] with partition p = d%128, k = d//128
    return np.ascontiguousarray(
        a.reshape(4, 128, cols).transpose(1, 0, 2).reshape(128, 4 * cols))


def _in_maps(x, Wg, Wout, sigma):
    bandA, bandB, bandC, ecb = _band_constants(sigma)
    wg = _pack_k(np.asarray(Wg, BF16), INNER)
    wo = np.ascontiguousarray(
        np.asarray(Wout, BF16).reshape(H, 128, D).transpose(1, 0, 2).reshape(128, H * D))
    x = np.asarray(x, np.float32)
    maps = []
    for c in range(8):
        b, s = divmod(c, 2)
        z = x[b] if s == 0 else x[b, ::-1]
        xbuf = np.zeros((VROWS, D), np.float32)
        xbuf[32:] = z[:1056]
        xT = _pack_k(np.ascontiguousarray(xbuf.T).astype(BF16), VROWS)
        maps.append({
            "xT": xT, "Wg": wg, "Wout": wo,
            "bandA": bandA, "bandB": bandB, "bandC": bandC, "ec": ecb,
        })
    return maps


def _get_nc():
    if "nc" not in _CACHE:
        _CACHE["nc"] = _build_nc()
    return _CACHE["nc"]


def run_spmd(in_maps, **kw):
    from concourse.bass_utils import run_bass_kernel_spmd
    return run_bass_kernel_spmd(_get_nc(), in_maps, core_ids=list(range(8)), **kw)


def _assemble(results):
    full = np.empty((B, N, D), np.float32)
    for c in range(8):
        b, s = divmod(c, 2)
        r = results[c]["out"]          # [128, 4096] bf16
        r = r.astype(np.float32).reshape(128, 4, 2, 512)
        r = r.transpose(1, 2, 0, 3).reshape(1024, 512)
        if s == 0:
            full[b, :1024] = r
        else:
            full[b, 1024:] = r[::-1]
    return full


def kernel(x, Wg, Wout, sigma):
    res = run_spmd(_in_maps(x, Wg, Wout, sigma))
    return _assemble(res.results)


# revision 11
# speedup vs baseline: 1.2319x; 1.1346x over previous
"""Local (Gaussian-banded) attention kernel for Trainium2, 8 NeuronCores.

Math: out = rownorm(gauss_band(sigma)) @ (x @ Wg) @ Wout
The Gaussian positional mask with sigma in [0.5, 2.5] decays below fp32
resolution past |i-j| > 32, so attention is a 65-tap banded matmul.

Sharding: core c = (batch b = c//2, seq-half s = c%2). Each core computes
out rows [s*1024, (s+1)*1024) of its batch. s=1 halves are row-reversed on
host so the sequence edge is always at local row 0 -> all 8 cores run the
same program with the same band constants (pure SPMD).

v3: all-bf16 datapath (fp32 PSUM), host-prepacked contiguous DMAs, and
redundant-LDWEIGHTS stripping. The PE weight-load scoreboard only lets
LDWEIGHTS k+2 issue after matmul k fully drains, so back-to-back matmuls
that each reload the stationary pace at ~(mm+ldw)/2 instead of the
stream rate. Consecutive matmuls that reuse one loaded stationary (the
redundant InstLdweights is deleted pre-compile; validated bit-exact on
HW) run at full stream rate:
  stage 1: for each (t,k): one LDW of the xT slice feeds the psA and
           psB matmuls (N=512 each).
  stage 2: 128-col output chunks, window = v[j] (K=128) + v[j+1][:64]
           (K=64): one LDW of v[j] feeds chunk j's A-matmul and chunk
           j-1's C-matmul (both N=128). Interior row-norm is baked into
           the two 128x128 band matrices (same for every chunk); the 32
           sequence-edge rows are fixed by a column rescale on the
           PSUM->SBUF cast output.
  stage 3: out[256,512] = attn chunk @ Wout (8 accumulating N=512
           matmuls; already stream-bound).
Measured rel err vs fp32 reference ~4.5e-3.
"""

import sys

for _p in ("/opt/trn_rl_repo", "/root/.axon_site/_ro/trn_rl_repo"):
    if _p not in sys.path:
        sys.path.append(_p)

import numpy as np
import ml_dtypes

BF16 = ml_dtypes.bfloat16

B, N, D = 4, 2048, 512
H, DH = 8, 128
INNER = H * DH
W = 32                      # band half-width
VROWS = 1088                # 32 zero pad + 1024 own + 32 halo
NT = 9                      # v tiles: 8 x 128 + 1 x 64
REUSE_C = True              # strip LDW on C-pieces (K=64 subset reuse)
STRIP_ON = True             # master switch for LDW stripping
# consts tensor column map (elements, bf16): bandA2 | bandC2 | ec | wout
CA, CC, CE, CW = 0, H * 128, 2 * H * 128, 2 * H * 128 + H * 32
CTOT = CW + H * D

_CACHE = {}


def _build_nc():
    import concourse.mybir as mybir
    from concourse import bacc
    from concourse.tile import TileContext

    f32 = mybir.dt.float32
    bf = mybir.dt.bfloat16

    nc = bacc.Bacc(None, target_bir_lowering=False)
    strip = []

    def mm(out, lhsT, rhs, reuse=False, **kw):
        i = nc.tensor.matmul(out, lhsT, rhs, **kw)
        if reuse and STRIP_ON:
            strip.append(i.ins.name)
        return i

    xT = nc.dram_tensor("xT", [128, 4 * VROWS], bf, kind="ExternalInput")
    wg = nc.dram_tensor("Wg", [128, 4 * INNER], bf, kind="ExternalInput")
    consts = nc.dram_tensor("consts", [128, CTOT], bf, kind="ExternalInput")
    # out rows r = i*256 + half*128 + p  ->  cols i*1024 + half*512 + d
    out = nc.dram_tensor("out", [128, 4 * 1024], bf, kind="ExternalOutput")

    with TileContext(nc) as tc:
        with (
            tc.tile_pool(name="persist", bufs=1) as pp,
            tc.tile_pool(name="outs", bufs=2) as osp,
            tc.tile_pool(name="ps1", bufs=2, space="PSUM") as ps1,
            tc.tile_pool(name="ps2", bufs=2, space="PSUM") as ps2,
            tc.tile_pool(name="ps3", bufs=2, space="PSUM") as ps3,
        ):
            xT_sb = pp.tile([128, 4 * VROWS], bf, tag="xT", name="xT_sb")
            wg_sb = pp.tile([128, 4 * INNER], bf, tag="wg", name="wg_sb")
            cs = pp.tile([128, CTOT], bf, tag="consts", name="cs")
            # k=0 slices first (gate the first matmuls), then the rest big
            nc.sync.dma_start(out=xT_sb[:, 0:VROWS], in_=xT[:, 0:VROWS])
            nc.scalar.dma_start(out=wg_sb[:, 0:INNER], in_=wg[:, 0:INNER])
            nc.sync.dma_start(out=xT_sb[:, VROWS:4 * VROWS],
                              in_=xT[:, VROWS:4 * VROWS])
            nc.scalar.dma_start(out=wg_sb[:, INNER:4 * INNER],
                                in_=wg[:, INNER:4 * INNER])
            nc.scalar.dma_start(out=cs, in_=consts[:, :])

            v_sb = [pp.tile([128, INNER], bf, tag=f"v{t}", name=f"v{t}")
                    for t in range(NT)]
            # attnT: [dh=128, (h, out col 0..1024)] bf16
            attnT = pp.tile([128, H * 1024], bf, tag="attnT", name="attnT")

            def s1(t):
                rows = 128 if t < 8 else 64
                # one 2-bank tile so the A/B halves allocate atomically and
                # the scheduler cannot split the weight-sharing pairs
                psT = ps1.tile([128, 1024], f32, tag="s1", name=f"psT{t}")
                psA, psB = psT[:, 0:512], psT[:, 512:1024]
                for k in range(4):
                    lh = xT_sb[:, k * VROWS + t * 128: k * VROWS + t * 128 + rows]
                    mm(psA[:rows, :], lh, wg_sb[:, k * INNER:k * INNER + 512],
                       start=(k == 0), stop=(k == 3))
                    mm(psB[:rows, :], lh, wg_sb[:, k * INNER + 512:(k + 1) * INNER],
                       reuse=True, start=(k == 0), stop=(k == 3))
                nc.vector.tensor_copy(v_sb[t][:rows, :], psT[:rows, :])

            def s2(q):
                # sweep bank q: out cols [512q, 512q+512), all heads
                for h in range(H):
                    hs = slice(h * 128, (h + 1) * 128)
                    bk = ps2.tile([128, 512], f32, tag="s2", name=f"s2_{q}_{h}")
                    for j in range(4 * q, 4 * q + 4):
                        c = (j % 4) * 128
                        mm(bk[:, c:c + 128], v_sb[j][:, hs],
                           cs[:, CA + h * 128:CA + (h + 1) * 128],
                           start=(j % 4 == 0), stop=False)
                        if j % 4 > 0:
                            mm(bk[:, c - 128:c], v_sb[j][:64, hs],
                               cs[:64, CC + h * 128:CC + (h + 1) * 128],
                               reuse=REUSE_C, start=False, stop=False)
                    mm(bk[:, 384:512], v_sb[4 * q + 4][:64, hs],
                       cs[:64, CC + h * 128:CC + (h + 1) * 128],
                       start=False, stop=True)
                    at = attnT[:, h * 1024 + q * 512: h * 1024 + q * 512 + 512]
                    nc.vector.tensor_copy(at, bk)
                    if q == 0:
                        # edge rescale: first 32 sequence rows of this head
                        v3 = attnT[:, h * 1024: h * 1024 + 32]
                        nc.vector.tensor_mul(
                            v3, v3, cs[:, CE + h * 32: CE + (h + 1) * 32])

            def s3(i):
                ot = osp.tile([128, 1024], bf, tag="outt", name=f"ot{i}")
                for half in range(2):
                    ps = ps3.tile([128, 512], f32, tag="s3", name=f"ps3_{i}_{half}")
                    for h in range(H):
                        off = h * 1024 + i * 256 + half * 128
                        mm(ps, attnT[:, off:off + 128],
                           cs[:, CW + h * D: CW + (h + 1) * D],
                           start=(h == 0), stop=(h == 7))
                    nc.vector.tensor_copy(ot[:, half * 512:(half + 1) * 512], ps)
                nc.scalar.dma_start(out=out[:, i * 1024:(i + 1) * 1024], in_=ot)

            s1(0); s1(1); s1(2); s1(3); s1(4)
            s2(0)
            s1(5); s1(6)
            s3(0)
            s1(7); s1(8)
            s3(1)
            s2(1)
            s3(2)
            s3(3)

    # Strip redundant InstLdweights: simulate the tensor queue in final
    # block order tracking the loaded stationary; a marked matmul's own
    # LDW is deleted only when the currently-loaded weights already cover
    # it (same tensor/offset/cols, partition-count superset). The Tile
    # scheduler may reorder pairs, so coverage is checked, not assumed.
    import concourse.mybir as mybir
    names = set(strip)

    def sig(ap):
        p = list(ap.ap)
        return (ap.memref, ap.offset, tuple(p[1]), p[0][1], p[0][0])

    def covers(loaded, w):
        return (loaded is not None and loaded[0] == w[0] and loaded[1] == w[1]
                and loaded[2] == w[2] and loaded[4] == w[4]
                and w[3] <= loaded[3])

    removed = 0
    kept = 0
    for blk in nc.m.functions[0].blocks:
        insts = blk.instructions
        loaded = None
        pend = None          # (idx, sig) of LDW awaiting its matmul
        dels = []
        for idx in range(len(insts)):
            inst = insts[idx]
            if isinstance(inst, mybir.InstLdweights):
                si = inst.sync_info
                assert pend is None, "two LDWs with no matmul between"
                pend = (idx, sig(inst.ins[0]),
                        si is None or len(si.on_wait) == 0)
            elif isinstance(inst, mybir.InstMatmult):
                w = sig(inst.ins[1])
                if pend is not None:
                    assert pend[1] == w, (pend[1], w)
                    if inst.name in names and covers(loaded, w) and pend[2]:
                        dels.append(pend[0])
                        removed += 1
                    else:
                        loaded = pend[1]
                        if inst.name in names:
                            kept += 1
                    pend = None
                else:
                    assert covers(loaded, w), (loaded, w)
        for idx in reversed(dels):
            del insts[idx]
    if removed + kept:
        sys.stderr.write(f"ldw strip: removed {removed}, kept {kept}\n")
    nc.compile()
    return nc


def _band_constants(sigma: np.ndarray):
    """Unified band matrices (interior row-norm baked in) + edge rescale."""
    sig = np.asarray(sigma, np.float64).reshape(H)
    d = np.arange(W + 1, dtype=np.float64)
    wts = np.exp(-(d[None, :] ** 2) / (2.0 * sig[:, None] ** 2))  # [H, 33]
    tail = wts[:, 1:].sum(1)
    s_int = wts[:, 0] + 2.0 * tail

    r = np.arange(128)
    c = np.arange(128)
    # A: src = padded row 128j+r (pos 128j+r-32), out col 128j+c
    distA = np.abs(c[None, :] - r[:, None] + 32)          # [128 src, 128 out]
    # C: src = padded row 128(j+1)+r2, r2 in [0,64)
    r2 = np.arange(64)
    distC = np.abs(c[None, :] - 96 - r2[:, None])         # [64, 128]

    def bands(dist, m):
        msk = dist <= W
        wp = np.where(msk[None], wts[:, np.minimum(dist, W).astype(int)], 0.0)
        wp = wp / s_int[:, None, None]                    # [H, m, 128]
        return np.ascontiguousarray(
            wp.transpose(1, 0, 2).reshape(m, H * 128)).astype(BF16)

    bandA2 = bands(distA, 128)
    bandC2 = bands(distC, 64)

    # edge rowsum for out rows 0..31 (left-truncated gaussian)
    re = np.arange(32)
    cum = np.concatenate([np.zeros((H, 1)), np.cumsum(wts[:, 1:], 1)], 1)
    s_edge = wts[:, [0]] + cum[:, np.minimum(re, W)] + tail[:, None]  # [H, 32]
    ecv = (s_int[:, None] / s_edge).astype(np.float32)
    ecb = np.ascontiguousarray(
        np.broadcast_to(ecv.reshape(1, H * 32), (128, H * 32))).astype(BF16)
    return bandA2, bandC2, ecb


def _pack_k(a, cols):
    # [512, cols] -> [128, 4*cols] with partition p = d%128, k = d//128
    return np.ascontiguousarray(
        a.reshape(4, 128, cols).transpose(1, 0, 2).reshape(128, 4 * cols))


def _consts(Wg_unused, Wout, sigma):
    bandA2, bandC2, ecb = _band_constants(sigma)
    cs = np.zeros((128, CTOT), BF16)
    cs[:, CA:CC] = bandA2
    cs[:64, CC:CE] = bandC2
    cs[:, CE:CW] = ecb
    cs[:, CW:] = np.asarray(Wout, BF16).reshape(H, 128, D).transpose(1, 0, 2) \
        .reshape(128, H * D)
    return cs


def _in_maps(x, Wg, Wout, sigma):
    cs = _consts(None, Wout, sigma)
    wg = _pack_k(np.asarray(Wg, BF16), INNER)
    x = np.asarray(x, np.float32)
    maps = []
    for c in range(8):
        b, s = divmod(c, 2)
        z = x[b] if s == 0 else x[b, ::-1]
        xbuf = np.zeros((VROWS, D), np.float32)
        xbuf[32:] = z[:1056]
        xT = _pack_k(np.ascontiguousarray(xbuf.T).astype(BF16), VROWS)
        maps.append({"xT": xT, "Wg": wg, "consts": cs})
    return maps


def _get_nc():
    if "nc" not in _CACHE:
        _CACHE["nc"] = _build_nc()
    return _CACHE["nc"]


def run_spmd(in_maps, **kw):
    from concourse.bass_utils import run_bass_kernel_spmd
    return run_bass_kernel_spmd(_get_nc(), in_maps, core_ids=list(range(8)), **kw)


def _assemble(results):
    full = np.empty((B, N, D), np.float32)
    for c in range(8):
        b, s = divmod(c, 2)
        r = results[c]["out"]          # [128, 4096] bf16
        r = r.astype(np.float32).reshape(128, 4, 2, 512)
        r = r.transpose(1, 2, 0, 3).reshape(1024, 512)
        if s == 0:
            full[b, :1024] = r
        else:
            full[b, 1024:] = r[::-1]
    return full


def kernel(x, Wg, Wout, sigma):
    res = run_spmd(_in_maps(x, Wg, Wout, sigma))
    return _assemble(res.results)
